# revision 2
# baseline (speedup 1.0000x reference)
"""Trainium2 Bass kernel for nn_AFFN (autoregressive FFN, block-triangular weights).

Math: the reference's sequential scan is only apparently sequential -- causality
is in the (already masked) block-triangular weights, so the whole model is three
dense feed-forward matmuls with elu between them:
    h0 = x_in_onehot @ W0f + b0 ; a1 = elu(h0)+1   (shifted elu; the +1 shift is
    h1 = a1 @ W1f + b1'        ; a2 = elu(h1)+1     compensated host-side via
    h2 = a2 @ W2f + b2'        ; out = elu(h2)      b' = b - sum_k Wq[k])
    logp[b] = sum_j log_softmax(out[b,j,:])[x[b,j]]

Precision/speed: all matmuls run in fp8e4m3 with the DoubleRow perf mode
(2 contraction k-tiles per pass, 2 fp8 MACs/cell/cycle = 2x bf16 rate, 112ns
per [256K x 128M x 256N] instruction, ldweights fully hidden).  Weights are
quantized host-side; activations are quantized by the elu chain's final
vector op writing fp8 directly.  Triangular structure: layer-1/2 only load and
multiply k-pairs t <= jb//2 (zero blocks skipped).

elu(x)+1 = min(exp(x),1) + max(x,0): per tile one ACT exp + one relu (split
ACT/DVE for engine balance) + one DVE combine into the fp8 pair tile.

Sharding: pure data parallel, batch 4096 -> 512 per core; weights replicated.

PSUM budget (8 banks, one open accumulation group per bank max):
  4 shared (layer-0 + layer-1 + epilogue transposes), 4 layer-2 (tout x half).
"""

import numpy as np
import ml_dtypes

L = 64
H = 64
IN = 4
B = 4096
NCORES = 8
BS = B // NCORES          # 512 batch rows per core
NJB = 32                  # feature tiles of 128 = 2 sites x 64
NPAIR = 16                # kb-tile pairs for DoubleRow

_CACHE = {}

# host-side W1 slab offsets (bytes per partition): group g8 has 2*g8+2 pairs
_W1_SLAB = [(2 * g + 2) * 1024 for g in range(8)]
_W1_OFF = np.concatenate([[0], np.cumsum(_W1_SLAB)]).astype(int)
W1R_COLS = int(_W1_OFF[-1])           # 73728


def _build():
    import concourse.tile as tile
    import concourse.mybir as mybir
    from concourse import bacc
    from concourse.masks import make_identity

    f32 = mybir.dt.float32
    bf16 = mybir.dt.bfloat16
    f8 = mybir.dt.float8e4
    Exp = mybir.ActivationFunctionType.Exp
    Ln = mybir.ActivationFunctionType.Ln
    Relu = mybir.ActivationFunctionType.Relu
    add = mybir.AluOpType.add
    amax = mybir.AluOpType.max
    amin = mybir.AluOpType.min
    mult = mybir.AluOpType.mult
    subtract = mybir.AluOpType.subtract
    DR = mybir.MatmulPerfMode.DoubleRow

    nc = bacc.Bacc("TRN2", target_bir_lowering=False, debug=False)

    x1h = nc.dram_tensor("x1h", [128, 2, BS], f8, kind="ExternalInput").ap()
    W0R = nc.dram_tensor("W0R", [128, 2, NJB, 128], f8, kind="ExternalInput").ap()
    W1R = nc.dram_tensor("W1R", [128, W1R_COLS], f8, kind="ExternalInput").ap()
    W2R = nc.dram_tensor("W2R", [128, NPAIR, 2, 2, 128], f8,
                         kind="ExternalInput").ap()
    b0c = nc.dram_tensor("b0c", [128, 32], f32, kind="ExternalInput").ap()
    b1c = nc.dram_tensor("b1c", [128, 32], f32, kind="ExternalInput").ap()
    b2c = nc.dram_tensor("b2c", [128, 2], f32, kind="ExternalInput").ap()
    mk0 = nc.dram_tensor("mk0", [128, 4, 32, 4], bf16, kind="ExternalInput").ap()
    mk1 = nc.dram_tensor("mk1", [128, 4, 32, 4], bf16, kind="ExternalInput").ap()
    out = nc.dram_tensor("out", [BS], f32, kind="ExternalOutput").ap()

    with tile.TileContext(nc) as tc:
        with (
            tc.tile_pool(name="singles", bufs=1) as singles,
            tc.tile_pool(name="w1p", bufs=2) as w1p,
            tc.tile_pool(name="a2p", bufs=2) as a2p,
            tc.tile_pool(name="tmp", bufs=6) as tmp,
            tc.tile_pool(name="epi", bufs=1) as epi,
            tc.tile_pool(name="psA", bufs=4, space="PSUM") as psA,
            tc.tile_pool(name="ps2", bufs=1, space="PSUM") as ps2,
        ):
            # ---- staged constants / inputs ----
            x1sb = singles.tile([128, 2, BS], f8)
            nc.sync.dma_start(x1sb[:], x1h)
            w0sb = singles.tile([128, 2, NJB, 128], f8)
            nc.sync.dma_start(w0sb[:], W0R)
            w2sb = singles.tile([128, NPAIR, 2, 2, 128], f8)
            nc.scalar.dma_start(w2sb[:], W2R)
            b0sb = singles.tile([128, 32], f32)
            nc.scalar.dma_start(b0sb[:], b0c)
            b1sb = singles.tile([128, 32], f32)
            nc.scalar.dma_start(b1sb[:], b1c)
            b2sb = singles.tile([128, 2], f32)
            nc.scalar.dma_start(b2sb[:], b2c)

            a1pairs = [singles.tile([128, 2, BS], f8, name=f"a1_{t}")
                       for t in range(NPAIR)]

            # layer-2 psum accumulators: one bank per (tout, half); only
            # [:, 0:256] of each is used -- full-bank tiles keep the two open
            # accumulation groups in separate banks (HW requirement).
            psum2 = {(t, h): ps2.tile([128, BS], f32, name=f"ps2_{t}{h}")
                     for t in range(2) for h in range(2)}

            epi_consts = {}

            def load_epi_consts():
                if epi_consts:
                    return
                ident = singles.tile([128, 128], f32, name="ident")
                make_identity(nc, ident[:])
                mks = []
                for t, dram in ((0, mk0), (1, mk1)):
                    mk = singles.tile([128, 4, 32, 4], bf16, name=f"mk{t}")
                    nc.scalar.dma_start(mk[:], dram)
                    mks.append(mk)
                epi_consts.update(ident=ident, mks=mks)

            def elu_chain(psum, bcol, out_slice, relu_on_act):
                """out_slice(fp8) = min(exp(psum+bcol),1) + max(psum+bcol,0)."""
                e = tmp.tile([128, BS], bf16, name="e_t", tag="e_t")
                nc.scalar.activation(e[:], psum[:], Exp, bias=bcol, scale=1.0)
                r = tmp.tile([128, BS], bf16, name="r_t", tag="r_t")
                if relu_on_act:
                    nc.scalar.activation(r[:], psum[:], Relu, bias=bcol,
                                         scale=1.0)
                else:
                    nc.vector.tensor_scalar(out=r[:], in0=psum[:], scalar1=bcol,
                                            scalar2=0.0, op0=add, op1=amax)
                nc.vector.scalar_tensor_tensor(out=out_slice, in0=e[:],
                                               scalar=1.0, in1=r[:],
                                               op0=amin, op1=add)

            emitted_l0 = [0]

            def emit_layer0_upto(jb_max):
                while emitted_l0[0] <= min(jb_max, NJB - 1):
                    jb = emitted_l0[0]
                    p0 = psA.tile([128, BS], f32, name="p0", tag="psA")
                    for h in range(2):
                        nc.tensor.matmul(
                            p0[:, 256 * h:256 * (h + 1)],
                            w0sb[:, :, jb, :],
                            x1sb[:, :, 256 * h:256 * (h + 1)],
                            start=True, stop=True, perf_mode=DR)
                    elu_chain(p0, b0sb[:, jb:jb + 1],
                              a1pairs[jb // 2][:, jb % 2, :],
                              relu_on_act=(jb % 2 == 0))
                    emitted_l0[0] += 1

            lpacc = singles.tile([128, BS // 128], f32)

            def emit_epilogue_half(t):
                """log-softmax for j in [32t, 32t+32) from psum2[(t, h)]."""
                ident, mks = epi_consts["ident"], epi_consts["mks"]
                h_ = epi.tile([128, BS], f32, name=f"hb{t}", tag=f"hb{t}")
                for hh in range(2):
                    nc.vector.tensor_scalar(
                        out=h_[:, 256 * hh:256 * (hh + 1)],
                        in0=psum2[(t, hh)][:, 0:256],
                        scalar1=b2sb[:, t:t + 1], scalar2=None, op0=add)
                oT = epi.tile([128, 4, 128], f32, name="oT", tag="oT")
                for c in range(4):
                    ptr = psA.tile([128, BS], f32, name="ptr", tag="psA")
                    nc.tensor.transpose(
                        ptr[:, 0:128], h_[:, 128 * c:128 * (c + 1)], ident[:])
                    nc.vector.tensor_copy(oT[:, c, :], ptr[:, 0:128])
                flat = oT[:].rearrange("p c f -> p (c f)")
                oc = epi.tile([128, 512], f32, name="oc", tag="oc")
                nc.vector.tensor_scalar(
                    out=oc[:], in0=flat, scalar1=80.0, scalar2=None, op0=amin)
                e = epi.tile([128, 512], f32, name="e_ep", tag="e_ep")
                nc.scalar.activation(e[:], oc[:], Exp)
                t1 = epi.tile([128, 512], f32, name="t1_ep", tag="t1_ep")
                nc.vector.tensor_scalar(
                    out=t1[:], in0=e[:], scalar1=1.0, scalar2=-1.0,
                    op0=amin, op1=add)
                v = epi.tile([128, 4, 32, 4], f32, name="v_ep", tag="v_ep")
                nc.vector.scalar_tensor_tensor(
                    out=v[:].rearrange("p c j s -> p (c j s)"), in0=flat,
                    scalar=0.0, in1=t1[:], op0=amax, op1=add)
                m = epi.tile([128, 4, 32], f32, name="m_ep", tag="m_ep")
                nc.vector.tensor_reduce(
                    out=m[:], in_=v[:], axis=mybir.AxisListType.X, op=amax)
                z = epi.tile([128, 4, 32, 4], f32, name="z_ep", tag="z_ep")
                nc.vector.tensor_tensor(
                    z[:], v[:], m[:, :, :, None].to_broadcast((128, 4, 32, 4)),
                    subtract)
                E = epi.tile([128, 4, 32, 4], f32, name="E_ep", tag="E_ep")
                nc.scalar.activation(E[:].rearrange("p c j s -> p (c j s)"),
                                     z[:].rearrange("p c j s -> p (c j s)"), Exp)
                S = epi.tile([128, 4, 32], f32, name="S_ep", tag="S_ep")
                nc.vector.tensor_reduce(
                    out=S[:], in_=E[:], axis=mybir.AxisListType.X, op=add)
                Lg = epi.tile([128, 4, 32], f32, name="Lg_ep", tag="Lg_ep")
                nc.scalar.activation(
                    Lg[:].rearrange("p c j -> p (c j)"),
                    S[:].rearrange("p c j -> p (c j)"), Ln)
                vm = epi.tile([128, 4, 32, 4], f32, name="vm_ep", tag="vm_ep")
                nc.vector.tensor_tensor(vm[:], z[:], mks[t][:], mult)
                selz = epi.tile([128, 4, 32], f32, name="selz_ep", tag="selz_ep")
                nc.vector.tensor_reduce(
                    out=selz[:], in_=vm[:], axis=mybir.AxisListType.X, op=add)
                d = epi.tile([128, 4, 32], f32, name="d_ep", tag="d_ep")
                nc.vector.tensor_tensor(d[:], selz[:], Lg[:], subtract)
                if t == 0:
                    nc.vector.tensor_reduce(
                        out=lpacc[:], in_=d[:], axis=mybir.AxisListType.X, op=add)
                else:
                    lp1 = epi.tile([128, 4], f32, name="lp1", tag="lp1")
                    nc.vector.tensor_reduce(
                        out=lp1[:], in_=d[:], axis=mybir.AxisListType.X, op=add)
                    nc.vector.tensor_add(lpacc[:], lpacc[:], lp1[:])
                    nc.sync.dma_start(
                        out.rearrange("(c p) -> p c", p=128), lpacc[:])

            # ---- main pipeline over 8 groups of 4 jb (8 j-sites) ----
            a2cur = [None]
            for g8 in range(8):
                T_g = 2 * g8 + 2
                w1g = w1p.tile([128, T_g, 2, 512], f8, name="w1g", tag="w1g")
                nc.sync.dma_start(
                    w1g[:].rearrange("p t i c -> p (t i c)"),
                    W1R[:, _W1_OFF[g8]:_W1_OFF[g8 + 1]])

                if g8 == 3:
                    load_epi_consts()

                emit_layer0_upto(4 * g8 + (3 if g8 == 0 else 7))

                for m_ in range(4):
                    jb = 4 * g8 + m_
                    tmax = jb // 2
                    p1 = psA.tile([128, BS], f32, name="p1", tag="psA")
                    for h in range(2):
                        for t in range(tmax + 1):
                            nc.tensor.matmul(
                                p1[:, 256 * h:256 * (h + 1)],
                                w1g[:, t, :, 128 * m_:128 * (m_ + 1)],
                                a1pairs[t][:, :, 256 * h:256 * (h + 1)],
                                start=(t == 0), stop=(t == tmax),
                                perf_mode=DR)
                    if jb % 2 == 0:
                        a2cur[0] = a2p.tile([128, 2, BS], f8, name="a2",
                                            tag="a2")
                    elu_chain(p1, b1sb[:, jb:jb + 1],
                              a2cur[0][:, jb % 2, :],
                              relu_on_act=(jb % 2 == 1))
                    if jb % 2 == 1:
                        t = jb // 2
                        for tout in range(2):
                            if t >= 8 and tout == 0:
                                continue
                            tstop = 7 if tout == 0 else 15
                            for h in range(2):
                                nc.tensor.matmul(
                                    psum2[(tout, h)][:, 0:256],
                                    w2sb[:, t, :, tout, :],
                                    a2cur[0][:, :, 256 * h:256 * (h + 1)],
                                    start=(t == 0), stop=(t == tstop),
                                    perf_mode=DR)

                if g8 == 3:
                    emit_epilogue_half(0)
            emit_epilogue_half(1)

    nc.compile()
    return nc


def _host_prep(x, W0, W1, W2, b0, b1, b2):
    f8 = ml_dtypes.float8_e4m3
    x = np.ascontiguousarray(np.asarray(x, dtype=np.int32))
    W0 = np.asarray(W0, dtype=np.float32)
    W1 = np.asarray(W1, dtype=np.float32)
    W2 = np.asarray(W2, dtype=np.float32)
    b0 = np.asarray(b0, dtype=np.float64)
    b1 = np.asarray(b1, dtype=np.float64)
    b2 = np.asarray(b2, dtype=np.float64)

    W0q = W0.astype(f8)
    W1q = W1.astype(f8)
    W2q = W2.astype(f8)

    # DoubleRow stationary layouts (see _build header for index math)
    # W0R[p=(il,ks), t, jb, m=(jp,s)] = W0q[ks, 2t+il, 2jb+jp, s]
    W0R = np.ascontiguousarray(
        W0q.reshape(64, 2, 2, 32, 2, 64)         # ks, t, il, jb, jp, s
           .transpose(2, 0, 1, 3, 4, 5)          # il, ks, t, jb, jp, s
           .reshape(128, 2, 32, 128))
    # W1R[p=(kp,i), t, i2, jb, m=(jp,s)] = W1q[4t+2*i2+kp, i, 2jb+jp, s]
    R1 = (W1q.reshape(16, 2, 2, 64, 32, 2, 64)   # t, i2, kp, i, jb, jp, s
             .transpose(2, 3, 0, 1, 4, 5, 6)     # kp, i, t, i2, jb, jp, s
             .reshape(128, 16, 2, 32, 128))
    slabs = [np.ascontiguousarray(
        R1[:, :2 * g + 2, :, 4 * g:4 * g + 4, :]).reshape(128, -1)
        for g in range(8)]
    W1R = np.ascontiguousarray(np.concatenate(slabs, axis=1))
    assert W1R.shape == (128, W1R_COLS)
    # W2R[p=(kp,i), t, i2, tout, m=(mj,s4)] = W2q[4t+2*i2+kp, i, 32*tout+mj, s4]
    W2R = np.ascontiguousarray(
        W2q.reshape(16, 2, 2, 64, 2, 32, 4)      # t, i2, kp, i, tout, mj, s4
           .transpose(2, 3, 0, 1, 4, 5, 6)
           .reshape(128, 16, 2, 2, 128))

    # shifted-elu compensation: b' = b - sum_k Wq[k] (device-exact weights)
    c1 = W1q.astype(np.float64).sum(axis=(0, 1))          # [64, 64]
    c2 = W2q.astype(np.float64).sum(axis=(0, 1))          # [64, 4]
    b1p = b1 - c1
    b2p = b2 - c2

    # bias columns: bc[p, jb] = b[2jb + p//64, p%64]
    b0c_ = np.ascontiguousarray(
        b0.reshape(4096).reshape(32, 128).T.astype(np.float32))
    b1c_ = np.ascontiguousarray(
        b1p.reshape(4096).reshape(32, 128).T.astype(np.float32))
    b2c_ = np.ascontiguousarray(
        b2p.reshape(256).reshape(2, 128).T.astype(np.float32))

    in_maps = []
    for c in range(NCORES):
        xs = x[c * BS:(c + 1) * BS]                       # (BS, L)
        # shifted input: site k sees one-hot of x[k-1]; site 0 sees zeros
        xt = np.full((L, BS), -1, dtype=np.int32)
        xt[1:] = xs.T[: L - 1]
        # x1h[p=(il,ks), t, b] = (xt[ks, b] == 2t + il)
        vals = 2 * np.arange(2)[None, :, None, None] + \
            np.arange(2)[:, None, None, None]             # il, t, 1, 1
        x1h = (xt[None, None, :, :] == vals).astype(f8)   # il, t, ks, b
        x1h = np.ascontiguousarray(
            x1h.transpose(0, 2, 1, 3).reshape(128, 2, BS))
        # epilogue select masks: mk[p, c4, j, s] = (xs[128*c4+p, 32t+j] == s)
        mks = []
        for t in range(2):
            sel = xs.reshape(4, 128, 64)[:, :, 32 * t:32 * t + 32]  # c4, p, j
            mk = (sel[:, :, :, None] == np.arange(4)[None, None, None, :])
            mks.append(np.ascontiguousarray(
                mk.transpose(1, 0, 2, 3).astype(ml_dtypes.bfloat16)))
        in_maps.append({
            "x1h": x1h, "W0R": W0R, "W1R": W1R, "W2R": W2R,
            "b0c": b0c_, "b1c": b1c_, "b2c": b2c_,
            "mk0": mks[0], "mk1": mks[1],
        })
    return in_maps


def _run(in_maps, trace=False, **kw):
    from concourse.bass_utils import run_bass_kernel_spmd
    if "nc" not in _CACHE:
        _CACHE["nc"] = _build()
    return run_bass_kernel_spmd(
        _CACHE["nc"], in_maps, core_ids=list(range(NCORES)), trace=trace, **kw)


def kernel(x, W0, W1, W2, b0, b1, b2):
    in_maps = _host_prep(x, W0, W1, W2, b0, b1, b2)
    res = _run(in_maps)
    return np.concatenate([r["out"] for r in res.results]).astype(np.float32)
